# revision 9
# baseline (speedup 1.0000x reference)
"""Trainium2 Bass kernel for the dual-branch agent-attention module.

Sharding: data-parallel over B=8 (one batch element per NeuronCore).
All transposes and weight permutations are done host-side; on-device
work is a streamed bf16 pipeline.

Math restructuring vs the reference:
  - Effective score weights Weff_A = Wq @ k12bd and Weff_B = Wkhf @ qabd
    (associativity: the big activations never materialize q or kh).
  - Scalar softmax biases ba/bb cancel (softmax shift invariance) and
    are dropped. Branch-A's per-agent bias c_A = k12bd^T @ bq survives
    as the exp's per-partition bias.
  - v bias bv is folded in AFTER the xs softmax-normalize
    (xs_n = xs0/denom + bv), so the per-tile v-bias matmul disappears;
    the softmax denominators come from ones columns memset into the
    v tile.
  - proj bias is added host-side.

Dataflow (per core):
  stage 1 (per 512-col chunk of N):
    B: v = attnT^T@Wv, scores t = attnT^T@Weff_B (wide 512/256 rhs,
       stationary operand shared k-major), exp on ACT straight from
       PSUM, xs accumulated in PSUM across all 32 seq tiles (a single
       K=1 zeroing matmul opens the accumulation region).
    AC: scores s = Weff_A^T@xT per head pair, exp(+c_A) into a
       persistent SBUF pa buffer.
  stage 1.5: xs normalize -> block-diag [xs | 1] tiles.
  stage 2 (per seq tile): x_out = PA^T @ xs_bd with ones-column
    denominators, normalize, PE-transpose, proj, store.
"""

import os
import sys
import numpy as np

for _p in ("/opt/trn_rl_repo", os.path.expanduser("~/.axon_site/_ro/trn_rl_repo")):
    if os.path.isdir(_p) and _p not in sys.path:
        sys.path.insert(0, _p)

import ml_dtypes

import concourse.bass as bass
import concourse.bacc as bacc
import concourse.tile as tile
from concourse import mybir
from concourse.bass_utils import run_bass_kernel_spmd
from concourse.masks import make_identity

BF16 = mybir.dt.bfloat16
F32 = mybir.dt.float32
NPBF16 = ml_dtypes.bfloat16

B, N, NA, H, D = 8, 4096, 64, 12, 32
C = H * D            # 384
C2 = 2 * C           # 768
NP = H // 2          # 6 head pairs
CH = 512             # seq chunk
NCH = N // CH        # 8
TPC = CH // 128      # 4 seq tiles per chunk
SCALE = D ** -0.5

_CACHE = {}


def _build_bass(finalize=True, zero_bias=False):
    nc = bacc.Bacc()

    # ---- DRAM I/O ----
    xT = nc.dram_tensor("xT", [C, N], BF16, kind="ExternalInput")
    attnT = nc.dram_tensor("attnT", [C, N], BF16, kind="ExternalInput")
    agT = nc.dram_tensor("agT", [C, NA], BF16, kind="ExternalInput")
    wqT = nc.dram_tensor("wqT", [C2, C], BF16, kind="ExternalInput")
    wkag = nc.dram_tensor("wkag", [C, C2], BF16, kind="ExternalInput")
    wqag = nc.dram_tensor("wqag", [C, C2], BF16, kind="ExternalInput")
    wkhfT = nc.dram_tensor("wkhfT", [C2, C], BF16, kind="ExternalInput")
    wv = nc.dram_tensor("wv", [C, C], BF16, kind="ExternalInput")
    wproj = nc.dram_tensor("wproj", [C, C], BF16, kind="ExternalInput")
    if not zero_bias:
        bq = nc.dram_tensor("bq", [C2], F32, kind="ExternalInput")
        bkag = nc.dram_tensor("bkag", [C2], F32, kind="ExternalInput")
        bqag = nc.dram_tensor("bqag", [C2], F32, kind="ExternalInput")
        bvh = nc.dram_tensor("bvh", [2 * NP * D], F32, kind="ExternalInput")
    out = nc.dram_tensor("out", [N, C], BF16, kind="ExternalOutput")

    Exp = mybir.ActivationFunctionType.Exp

    with tile.TileContext(nc) as tc:
        with (
            tc.tile_pool(name="const", bufs=1) as const,
            tc.tile_pool(name="vv", bufs=2) as p_v,
            tc.tile_pool(name="pt", bufs=3) as p_pt,
            tc.tile_pool(name="xon", bufs=2) as p_xon,
            tc.tile_pool(name="xot", bufs=3) as p_xot,
            tc.tile_pool(name="osb", bufs=3) as p_out,
            tc.tile_pool(name="sm", bufs=4) as p_sm,
            tc.tile_pool(name="psA", bufs=4, space="PSUM") as psA,
            tc.tile_pool(name="psC", bufs=3, space="PSUM") as psC,
            tc.tile_pool(name="psX", bufs=1, space="PSUM") as psX,
        ):
            # ---- constants ----
            w_qT = const.tile([128, 6, C], BF16)
            w_khfT = const.tile([128, 6, C], BF16)
            w_kag = const.tile([128, 3, C2], BF16)
            w_qag = const.tile([128, 3, C2], BF16)
            w_v = const.tile([128, 3, C], BF16)
            w_pr = const.tile([128, 3, C], BF16)
            for dst, src in ((w_kag, wkag), (w_qag, wqag), (w_qT, wqT),
                             (w_khfT, wkhfT), (w_v, wv), (w_pr, wproj)):
                nc.sync.dma_start(out=dst, in_=src.rearrange("(k p) m -> p k m", p=128))
            ag_t = const.tile([128, 3, NA], BF16)
            nc.gpsimd.dma_start(out=ag_t, in_=agT.rearrange("(k p) m -> p k m", p=128))
            if not zero_bias:
                b_q = const.tile([128, 6], F32)
                b_kag = const.tile([128, 6], F32)
                b_qag = const.tile([128, 6], F32)
                for dst, src in ((b_q, bq), (b_kag, bkag), (b_qag, bqag)):
                    nc.gpsimd.dma_start(out=dst, in_=src.rearrange("(j p) -> p j", p=128))
                bvb = const.tile([128, NP, D], F32)
                nc.gpsimd.dma_start(
                    out=bvb[0:64],
                    in_=bass.AP(tensor=bvh[:].tensor, offset=0,
                                ap=[[0, 64], [1, NP * D]]))
                nc.gpsimd.dma_start(
                    out=bvb[64:128],
                    in_=bass.AP(tensor=bvh[:].tensor, offset=NP * D,
                                ap=[[0, 64], [1, NP * D]]))
            zrow = const.tile([1, 396], BF16)
            nc.vector.memset(zrow, 0.0)

            # full activations resident in SBUF
            at_full = const.tile([128, 3, N], BF16)
            xt_full = const.tile([128, 3, N], BF16)
            for c in range(NCH):
                nc.scalar.dma_start(
                    out=at_full[:, :, c * CH:(c + 1) * CH],
                    in_=attnT.rearrange("(k p) s -> p k s", p=128)[:, :, c * CH:(c + 1) * CH])
                nc.scalar.dma_start(
                    out=xt_full[:, :, c * CH:(c + 1) * CH],
                    in_=xT.rearrange("(k p) s -> p k s", p=128)[:, :, c * CH:(c + 1) * CH])
            pa_full = const.tile([128, 6, N], BF16)

            # Pre-touch DMA-loaded bias constants with tiny reads.
            if not zero_bias:
                touch = const.tile([128, 16], F32)
                for i, t_ap in enumerate((b_q[:, 0:1], b_kag[:, 0:1],
                                          b_qag[:, 0:1], bvb[:, 0:1, 0])):
                    nc.vector.tensor_copy(touch[:, i:i + 1], t_ap)

            # ---- prep: k_ag / qa projections -> block-diag tiles ----
            kag_sb = const.tile([128, 6, NA], BF16)
            qa_sb = const.tile([128, 6, NA], BF16)
            for w_t, b_t, dst in ((w_kag, "bkag", kag_sb), (w_qag, "bqag", qa_sb)):
                for j in range(6):
                    ps = psA.tile([128, NA], F32, tag="pA")
                    for k in range(3):
                        nc.tensor.matmul(ps, lhsT=w_t[:, k, j * 128:(j + 1) * 128],
                                         rhs=ag_t[:, k, :], start=(k == 0), stop=(k == 2))
                    if zero_bias:
                        nc.vector.tensor_copy(dst[:, j, :], ps)
                    else:
                        bt = b_kag if b_t == "bkag" else b_qag
                        nc.vector.tensor_add(dst[:, j, :], ps,
                                             bt[:, j:j + 1].to_broadcast([128, NA]))
            k12bd = const.tile([128, 6, 128], BF16)
            qabd = const.tile([128, 6, 128], BF16)
            for src, dst in ((kag_sb, k12bd), (qa_sb, qabd)):
                nc.vector.memset(dst, 0.0)
                for j in range(6):
                    nc.vector.tensor_copy(dst[0:64, j, 0:64], src[0:64, j, :])
                    nc.vector.tensor_copy(dst[64:128, j, 64:128], src[64:128, j, :])

            # ---- prep: effective score weights + branch-A exp bias ----
            weff_a = const.tile([128, 3, C2], BF16)
            weff_b = const.tile([128, 3, C2], BF16)
            for j in range(6):
                for k in range(3):
                    ps = psA.tile([128, 128], F32, tag="pA")
                    nc.tensor.matmul(ps, lhsT=w_qT[:, j, k * 128:(k + 1) * 128],
                                     rhs=k12bd[:, j, :], start=True, stop=True)
                    nc.vector.tensor_copy(weff_a[:, k, j * 128:(j + 1) * 128], ps)
                    ps2 = psA.tile([128, 128], F32, tag="pA")
                    nc.tensor.matmul(ps2, lhsT=w_khfT[:, j, k * 128:(k + 1) * 128],
                                     rhs=qabd[:, j, :], start=True, stop=True)
                    nc.scalar.copy(weff_b[:, k, j * 128:(j + 1) * 128], ps2)
            cba = None
            if not zero_bias:
                b_q_bf = const.tile([128, 6], BF16)
                nc.vector.tensor_copy(b_q_bf, b_q)
                cba = const.tile([128, 6], F32)
                for j in range(6):
                    ps = psA.tile([128, 1], F32, tag="pA")
                    nc.tensor.matmul(ps, lhsT=k12bd[:, j, :], rhs=b_q_bf[:, j:j + 1],
                                     start=True, stop=True)
                    nc.vector.tensor_copy(cba[:, j:j + 1], ps)

            # ---- xs accumulator: open the PSUM region with a zero matmul ----
            xs_acc = psX.tile([128, 6, 66], F32)
            nc.tensor.matmul(xs_acc[:, :, :], lhsT=zrow[:, 0:128], rhs=zrow[:, 0:396],
                             start=True, stop=False, skip_group_check=True)

            # ---- stage 1: values + branch-B attention + branch-A scores ----
            pending_xs = None
            for c in range(NCH):
                v_t = p_v.tile([128, TPC, H, 33], BF16)
                nc.vector.memset(v_t[:, :, :, 32], 1.0)
                for t in range(TPC):
                    s0 = c * CH + t * 128
                    psv = psC.tile([128, C], F32, tag="pC")
                    ps4 = psA.tile([128, 512], F32, tag="pA")
                    ps2 = psC.tile([128, 256], F32, tag="pC")
                    for k in range(3):
                        at_k = at_full[:, k, s0:s0 + 128]
                        nc.tensor.matmul(psv, lhsT=at_k, rhs=w_v[:, k, :],
                                         start=(k == 0), stop=(k == 2))
                        nc.tensor.matmul(ps4, lhsT=at_k, rhs=weff_b[:, k, 0:512],
                                         start=(k == 0), stop=(k == 2))
                        nc.tensor.matmul(ps2, lhsT=at_k, rhs=weff_b[:, k, 512:768],
                                         start=(k == 0), stop=(k == 2))
                    pt = p_pt.tile([128, 768], BF16)
                    nc.scalar.activation(pt[:, 0:512], ps4, Exp)
                    nc.scalar.activation(pt[:, 512:768], ps2, Exp)
                    nc.vector.tensor_copy(
                        v_t[:, t, :, 0:32],
                        psv[:].rearrange("p (h d) -> p h d", d=32))
                    if pending_xs is not None:
                        pending_xs()
                    last = (c == NCH - 1 and t == TPC - 1)

                    def make_xs(pt=pt, v_t=v_t, t=t, last=last):
                        def emit():
                            for j in range(6):
                                nc.tensor.matmul(
                                    xs_acc[:, j, :], lhsT=pt[:, j * 128:(j + 1) * 128],
                                    rhs=v_t[:, t, 2 * j:2 * j + 2, :],
                                    start=False, stop=(last and j == 5),
                                    skip_group_check=True)
                        return emit
                    pending_xs = make_xs()
                for j in range(6):
                    ps = psA.tile([128, CH], F32, tag="pA")
                    for k in range(3):
                        nc.tensor.matmul(ps, lhsT=weff_a[:, k, j * 128:(j + 1) * 128],
                                         rhs=xt_full[:, k, c * CH:(c + 1) * CH],
                                         start=(k == 0), stop=(k == 2))
                    nc.scalar.activation(
                        pa_full[:, j, c * CH:(c + 1) * CH], ps, Exp,
                        bias=(0.0 if zero_bias else cba[:, j:j + 1]))
            pending_xs()

            # ---- stage 1.5: xs normalize -> block-diag [xs | 1] tiles ----
            xs_bd = const.tile([128, 6, 66], BF16)
            nc.vector.memset(xs_bd, 0.0)
            nc.vector.memset(xs_bd[0:64, :, 32:33], 1.0)
            nc.vector.memset(xs_bd[64:128, :, 65:66], 1.0)
            rec6 = p_sm.tile([128, 6], F32, tag="rec")
            nc.vector.reciprocal(rec6[0:64, :], xs_acc[0:64, :, 32])
            nc.vector.reciprocal(rec6[64:128, :], xs_acc[64:128, :, 65])
            nc.vector.tensor_mul(xs_bd[0:64, :, 0:32], xs_acc[0:64, :, 0:32],
                                 rec6[0:64, :].unsqueeze(2).to_broadcast([64, 6, 32]))
            nc.vector.tensor_mul(xs_bd[64:128, :, 33:65], xs_acc[64:128, :, 33:65],
                                 rec6[64:128, :].unsqueeze(2).to_broadcast([64, 6, 32]))
            if not zero_bias:
                nc.vector.tensor_add(xs_bd[0:64, :, 0:32], xs_bd[0:64, :, 0:32],
                                     bvb[0:64])
                nc.vector.tensor_add(xs_bd[64:128, :, 33:65], xs_bd[64:128, :, 33:65],
                                     bvb[64:128])

            # ---- stage 2: branch-A attention + proj ----
            for c in range(NCH):
                for t in range(TPC):
                    s0 = c * CH + t * 128
                    xo = psC.tile([128, 396], F32, tag="pC")
                    for j in range(6):
                        nc.tensor.matmul(xo[:, j * 66:(j + 1) * 66],
                                         lhsT=pa_full[:, j, s0:s0 + 128],
                                         rhs=xs_bd[:, j, :],
                                         start=True, stop=True)
                    xo3 = xo[:].rearrange("p (h d) -> p h d", d=33)
                    rec = p_sm.tile([128, 12], F32, tag="rec12")
                    nc.vector.reciprocal(rec, xo3[:, :, 32])
                    xon = p_xon.tile([128, C], BF16)
                    nc.vector.tensor_mul(xon[:].rearrange("p (h d) -> p h d", d=32),
                                         xo3[:, :, 0:32],
                                         rec[:].unsqueeze(2).to_broadcast([128, 12, 32]))
                    pr = psA.tile([128, C], F32, tag="pA")
                    for f in range(3):
                        xot = p_xot.tile([128, 128], BF16)
                        nc.sync.dma_start(out=xot, in_=xon[:, f * 128:(f + 1) * 128],
                                          transpose=True)
                        nc.tensor.matmul(pr, lhsT=xot, rhs=w_pr[:, f, :],
                                         start=(f == 0), stop=(f == 2),
                                         skip_group_check=True)
                    o_sb = p_out.tile([128, C], BF16)
                    nc.scalar.copy(o_sb, pr)
                    nc.scalar.dma_start(out=out[s0:s0 + 128, :], in_=o_sb)
    if finalize:
        nc.finalize()
    return nc


def _prep_host(inputs):
    f32 = np.float32
    x = np.asarray(inputs["x"], f32)
    attn = np.asarray(inputs["attn"], f32)
    agent = np.asarray(inputs["agent_input"], f32)
    wa = np.asarray(inputs["wa"], f32)
    wb = np.asarray(inputs["wb"], f32)

    perm = np.empty(C2, np.int64)
    sva = np.empty(C2, f32)
    svb = np.empty(C2, f32)
    for h in range(H):
        for br in range(2):
            j0 = h * 64 + br * 32
            perm[j0:j0 + 32] = br * C + h * 32 + np.arange(32)
            sva[j0:j0 + 32] = wa[br] * SCALE
            svb[j0:j0 + 32] = wb[br] * SCALE

    wq_p = np.asarray(inputs["Wq_lf"], f32)[:, perm]
    bq_p = np.asarray(inputs["bq_lf"], f32)[perm]
    wkag_p = np.asarray(inputs["Wk_ag"], f32)[:, perm] * sva[None, :]
    bkag_p = np.asarray(inputs["bk_ag"], f32)[perm] * sva
    wqag_p = np.asarray(inputs["Wq_ag"], f32)[:, perm]
    bqag_p = np.asarray(inputs["bq_ag"], f32)[perm]
    wkhf_p = np.asarray(inputs["Wk_hf"], f32)[:, perm] * svb[None, :]

    zb = all(not np.any(np.asarray(inputs[k]))
             for k in ("bq_lf", "bk_ag", "bq_ag", "bk_hf", "bv_hf", "ba", "bb"))

    shared = {
        "wqT": np.ascontiguousarray(wq_p.T).astype(NPBF16),
        "wkhfT": np.ascontiguousarray(wkhf_p.T).astype(NPBF16),
        "wkag": wkag_p.astype(NPBF16),
        "wqag": wqag_p.astype(NPBF16),
        "wv": np.asarray(inputs["Wv_hf"], f32).astype(NPBF16),
        "wproj": np.asarray(inputs["Wproj"], f32).astype(NPBF16),
    }
    if not zb:
        bv_in = np.asarray(inputs["bv_hf"], f32)
        # bvh[half, j, d]: half 0 = head 2j, half 1 = head 2j+1
        bvh = np.empty((2, NP, D), f32)
        for j in range(NP):
            bvh[0, j, :] = bv_in[(2 * j) * D:(2 * j + 1) * D]
            bvh[1, j, :] = bv_in[(2 * j + 1) * D:(2 * j + 2) * D]
        shared.update({
            "bq": bq_p, "bkag": bkag_p, "bqag": bqag_p,
            "bvh": np.ascontiguousarray(bvh.reshape(-1)),
        })
    xT = np.ascontiguousarray(x.transpose(0, 2, 1)).astype(NPBF16)
    attnT = np.ascontiguousarray(attn.transpose(0, 2, 1)).astype(NPBF16)
    agT = np.ascontiguousarray(agent.transpose(0, 2, 1)).astype(NPBF16)
    in_maps = []
    for b in range(B):
        m = dict(shared)
        m["xT"] = xT[b]
        m["attnT"] = attnT[b]
        m["agT"] = agT[b]
        in_maps.append(m)
    return in_maps, zb


def kernel(**inputs):
    in_maps, zb = _prep_host(inputs)
    key = ("nc", zb)
    if key not in _CACHE:
        _CACHE[key] = _build_bass(zero_bias=zb)
    nc = _CACHE[key]
    res = run_bass_kernel_spmd(nc, in_maps, core_ids=list(range(B)))
    outs = np.stack([np.asarray(res.results[b]["out"], np.float32)
                     for b in range(B)], axis=0)
    if not zb:
        outs = outs + np.asarray(inputs["bproj"], np.float32)[None, None, :]
    return outs


# revision 12
# speedup vs baseline: 1.5435x; 1.5435x over previous
"""Trainium2 Bass kernel for the dual-branch agent-attention module.

Sharding: data-parallel over B=8 (one batch element per NeuronCore).
All transposes and weight permutations are done host-side; on-device
work is a streamed bf16 pipeline.

Math restructuring vs the reference:
  - Effective score weights Weff_A = Wq @ k12bd and Weff_B = Wkhf @ qabd
    (associativity: the big activations never materialize q or kh).
  - Scalar softmax biases ba/bb cancel (softmax shift invariance) and
    are dropped. Branch-A's per-agent bias c_A = k12bd^T @ bq survives
    as the exp's per-partition bias.
  - v bias bv is folded in AFTER the xs softmax-normalize
    (xs_n = xs0/denom + bv), so the per-tile v-bias matmul disappears;
    the softmax denominators come from ones columns memset into the
    v tile.
  - proj bias is added host-side.

Dataflow (per core):
  stage 1 (per 512-col chunk of N):
    B: v = attnT^T@Wv, scores t = attnT^T@Weff_B (wide 512/256 rhs,
       stationary operand shared k-major), exp on ACT straight from
       PSUM, xs accumulated in PSUM across all 32 seq tiles (a single
       K=1 zeroing matmul opens the accumulation region).
    AC: scores s = Weff_A^T@xT per head pair, exp(+c_A) into a
       persistent SBUF pa buffer.
  stage 1.5: xs normalize -> block-diag [xs | 1] tiles.
  stage 2 (per seq tile): x_out = PA^T @ xs_bd with ones-column
    denominators, normalize, PE-transpose, proj, store.
"""

import os
import sys
import numpy as np

for _p in ("/opt/trn_rl_repo", os.path.expanduser("~/.axon_site/_ro/trn_rl_repo")):
    if os.path.isdir(_p) and _p not in sys.path:
        sys.path.insert(0, _p)

import ml_dtypes

import concourse.bass as bass
import concourse.bacc as bacc
import concourse.tile as tile
from concourse import mybir
from concourse.bass_utils import run_bass_kernel_spmd
from concourse.masks import make_identity

BF16 = mybir.dt.bfloat16
F32 = mybir.dt.float32
NPBF16 = ml_dtypes.bfloat16

B, N, NA, H, D = 8, 4096, 64, 12, 32
C = H * D            # 384
C2 = 2 * C           # 768
NP = H // 2          # 6 head pairs
CH = 512             # seq chunk
NCH = N // CH        # 8
TPC = CH // 128      # 4 seq tiles per chunk
SCALE = D ** -0.5

_CACHE = {}


def _build_bass(finalize=True, zero_bias=False):
    nc = bacc.Bacc()

    # ---- DRAM I/O ----
    xT = nc.dram_tensor("xT", [C, N], BF16, kind="ExternalInput")
    attnT = nc.dram_tensor("attnT", [C, N], BF16, kind="ExternalInput")
    agT = nc.dram_tensor("agT", [C, NA], BF16, kind="ExternalInput")
    wqT = nc.dram_tensor("wqT", [C2, C], BF16, kind="ExternalInput")
    wkag = nc.dram_tensor("wkag", [C, C2], BF16, kind="ExternalInput")
    wqag = nc.dram_tensor("wqag", [C, C2], BF16, kind="ExternalInput")
    wkhfT = nc.dram_tensor("wkhfT", [C2, C], BF16, kind="ExternalInput")
    wv = nc.dram_tensor("wv", [C, C], BF16, kind="ExternalInput")
    wproj = nc.dram_tensor("wproj", [C, C], BF16, kind="ExternalInput")
    if not zero_bias:
        bq = nc.dram_tensor("bq", [C2], F32, kind="ExternalInput")
        bkag = nc.dram_tensor("bkag", [C2], F32, kind="ExternalInput")
        bqag = nc.dram_tensor("bqag", [C2], F32, kind="ExternalInput")
        bvh = nc.dram_tensor("bvh", [2 * NP * D], F32, kind="ExternalInput")
    out = nc.dram_tensor("out", [N, C], BF16, kind="ExternalOutput")

    Exp = mybir.ActivationFunctionType.Exp

    with tile.TileContext(nc) as tc:
        with (
            tc.tile_pool(name="const", bufs=1) as const,
            tc.tile_pool(name="vv", bufs=2) as p_v,
            tc.tile_pool(name="pt", bufs=3) as p_pt,
            tc.tile_pool(name="xon", bufs=2) as p_xon,
            tc.tile_pool(name="xot", bufs=3) as p_xot,
            tc.tile_pool(name="osb", bufs=3) as p_out,
            tc.tile_pool(name="sm", bufs=4) as p_sm,
            tc.tile_pool(name="psA", bufs=4, space="PSUM") as psA,
            tc.tile_pool(name="psC", bufs=3, space="PSUM") as psC,
            tc.tile_pool(name="psX", bufs=1, space="PSUM") as psX,
            # psA: 4 banks (ps4/AC-scores/pr), psC: 3 banks (v/ps2/xo/tp),
            # psX: 1 bank (xs accumulator) -> 8 banks total.
        ):
            # ---- constants ----
            w_qT = const.tile([128, 6, C], BF16)
            w_khfT = const.tile([128, 6, C], BF16)
            w_kag = const.tile([128, 3, C2], BF16)
            w_qag = const.tile([128, 3, C2], BF16)
            w_v = const.tile([128, 3, C], BF16)
            w_pr = const.tile([128, 3, C], BF16)
            for dst, src in ((w_kag, wkag), (w_qag, wqag), (w_qT, wqT),
                             (w_khfT, wkhfT), (w_v, wv), (w_pr, wproj)):
                nc.sync.dma_start(out=dst, in_=src.rearrange("(k p) m -> p k m", p=128))
            ag_t = const.tile([128, 3, NA], BF16)
            nc.gpsimd.dma_start(out=ag_t, in_=agT.rearrange("(k p) m -> p k m", p=128))
            if not zero_bias:
                b_q = const.tile([128, 6], F32)
                b_kag = const.tile([128, 6], F32)
                b_qag = const.tile([128, 6], F32)
                for dst, src in ((b_q, bq), (b_kag, bkag), (b_qag, bqag)):
                    nc.gpsimd.dma_start(out=dst, in_=src.rearrange("(j p) -> p j", p=128))
                bvb = const.tile([128, NP, D], F32)
                nc.gpsimd.dma_start(
                    out=bvb[0:64],
                    in_=bass.AP(tensor=bvh[:].tensor, offset=0,
                                ap=[[0, 64], [1, NP * D]]))
                nc.gpsimd.dma_start(
                    out=bvb[64:128],
                    in_=bass.AP(tensor=bvh[:].tensor, offset=NP * D,
                                ap=[[0, 64], [1, NP * D]]))
            ident = const.tile([128, 128], BF16)
            make_identity(nc, ident)
            zrow = const.tile([1, 396], BF16)
            nc.vector.memset(zrow, 0.0)

            # full activations resident in SBUF
            at_full = const.tile([128, 3, N], BF16)
            xt_full = const.tile([128, 3, N], BF16)
            for c in range(NCH):
                nc.scalar.dma_start(
                    out=at_full[:, :, c * CH:(c + 1) * CH],
                    in_=attnT.rearrange("(k p) s -> p k s", p=128)[:, :, c * CH:(c + 1) * CH])
                nc.scalar.dma_start(
                    out=xt_full[:, :, c * CH:(c + 1) * CH],
                    in_=xT.rearrange("(k p) s -> p k s", p=128)[:, :, c * CH:(c + 1) * CH])
            pa_full = const.tile([128, 6, N], BF16)

            # Pre-touch DMA-loaded bias constants with tiny reads.
            if not zero_bias:
                touch = const.tile([128, 16], F32)
                for i, t_ap in enumerate((b_q[:, 0:1], b_kag[:, 0:1],
                                          b_qag[:, 0:1], bvb[:, 0:1, 0])):
                    nc.vector.tensor_copy(touch[:, i:i + 1], t_ap)

            # ---- prep: k_ag / qa projections -> block-diag tiles ----
            kag_sb = const.tile([128, 6, NA], BF16)
            qa_sb = const.tile([128, 6, NA], BF16)
            for w_t, b_t, dst in ((w_kag, "bkag", kag_sb), (w_qag, "bqag", qa_sb)):
                for j in range(6):
                    ps = psA.tile([128, NA], F32, tag="pA")
                    for k in range(3):
                        nc.tensor.matmul(ps, lhsT=w_t[:, k, j * 128:(j + 1) * 128],
                                         rhs=ag_t[:, k, :], start=(k == 0), stop=(k == 2))
                    if zero_bias:
                        nc.vector.tensor_copy(dst[:, j, :], ps)
                    else:
                        bt = b_kag if b_t == "bkag" else b_qag
                        nc.vector.tensor_add(dst[:, j, :], ps,
                                             bt[:, j:j + 1].to_broadcast([128, NA]))
            k12bd = const.tile([128, 6, 128], BF16)
            qabd = const.tile([128, 6, 128], BF16)
            for src, dst in ((kag_sb, k12bd), (qa_sb, qabd)):
                nc.vector.memset(dst, 0.0)
                for j in range(6):
                    nc.vector.tensor_copy(dst[0:64, j, 0:64], src[0:64, j, :])
                    nc.vector.tensor_copy(dst[64:128, j, 64:128], src[64:128, j, :])

            # ---- prep: effective score weights + branch-A exp bias ----
            weff_a = const.tile([128, 3, C2], BF16)
            weff_b = const.tile([128, 3, C2], BF16)
            for j in range(6):
                for k in range(3):
                    ps = psA.tile([128, 128], F32, tag="pA")
                    nc.tensor.matmul(ps, lhsT=w_qT[:, j, k * 128:(k + 1) * 128],
                                     rhs=k12bd[:, j, :], start=True, stop=True)
                    nc.vector.tensor_copy(weff_a[:, k, j * 128:(j + 1) * 128], ps)
                    ps2 = psA.tile([128, 128], F32, tag="pA")
                    nc.tensor.matmul(ps2, lhsT=w_khfT[:, j, k * 128:(k + 1) * 128],
                                     rhs=qabd[:, j, :], start=True, stop=True)
                    nc.scalar.copy(weff_b[:, k, j * 128:(j + 1) * 128], ps2)
            cba = None
            if not zero_bias:
                b_q_bf = const.tile([128, 6], BF16)
                nc.vector.tensor_copy(b_q_bf, b_q)
                cba = const.tile([128, 6], F32)
                for j in range(6):
                    ps = psA.tile([128, 1], F32, tag="pA")
                    nc.tensor.matmul(ps, lhsT=k12bd[:, j, :], rhs=b_q_bf[:, j:j + 1],
                                     start=True, stop=True)
                    nc.vector.tensor_copy(cba[:, j:j + 1], ps)

            # ---- xs accumulator: open the PSUM region with a zero matmul ----
            xs_acc = psX.tile([128, 6, 66], F32)
            nc.tensor.matmul(xs_acc[:, :, :], lhsT=zrow[:, 0:128], rhs=zrow[:, 0:396],
                             start=True, stop=False, skip_group_check=True)

            # ---- stage 1: values + branch-B attention + branch-A scores ----
            pending_xs = None
            for c in range(NCH):
                v_t = p_v.tile([128, TPC, H, 33], BF16)
                nc.vector.memset(v_t[:, :, :, 32], 1.0)
                for t in range(TPC):
                    s0 = c * CH + t * 128
                    psv = psC.tile([128, C], F32, tag="pC")
                    ps4 = psA.tile([128, 512], F32, tag="pA")
                    ps2 = psC.tile([128, 256], F32, tag="pC")
                    for k in range(3):
                        at_k = at_full[:, k, s0:s0 + 128]
                        nc.tensor.matmul(psv, lhsT=at_k, rhs=w_v[:, k, :],
                                         start=(k == 0), stop=(k == 2))
                        nc.tensor.matmul(ps4, lhsT=at_k, rhs=weff_b[:, k, 0:512],
                                         start=(k == 0), stop=(k == 2))
                        nc.tensor.matmul(ps2, lhsT=at_k, rhs=weff_b[:, k, 512:768],
                                         start=(k == 0), stop=(k == 2))
                    pt = p_pt.tile([128, 768], BF16)
                    nc.scalar.activation(pt[:, 0:512], ps4, Exp)
                    nc.scalar.activation(pt[:, 512:768], ps2, Exp)
                    nc.vector.tensor_copy(
                        v_t[:, t, :, 0:32],
                        psv[:].rearrange("p (h d) -> p h d", d=32))
                    if pending_xs is not None:
                        pending_xs()
                    last = (c == NCH - 1 and t == TPC - 1)

                    def make_xs(pt=pt, v_t=v_t, t=t, last=last):
                        def emit():
                            for j in range(6):
                                nc.tensor.matmul(
                                    xs_acc[:, j, :], lhsT=pt[:, j * 128:(j + 1) * 128],
                                    rhs=v_t[:, t, 2 * j:2 * j + 2, :],
                                    start=False, stop=(last and j == 5),
                                    skip_group_check=True)
                        return emit
                    pending_xs = make_xs()
                for j in range(6):
                    ps = psA.tile([128, CH], F32, tag="pA")
                    for k in range(3):
                        nc.tensor.matmul(ps, lhsT=weff_a[:, k, j * 128:(j + 1) * 128],
                                         rhs=xt_full[:, k, c * CH:(c + 1) * CH],
                                         start=(k == 0), stop=(k == 2))
                    nc.scalar.activation(
                        pa_full[:, j, c * CH:(c + 1) * CH], ps, Exp,
                        bias=(0.0 if zero_bias else cba[:, j:j + 1]))
            pending_xs()

            # ---- stage 1.5: xs normalize -> block-diag [xs | 1] tiles ----
            xs_bd = const.tile([128, 6, 66], BF16)
            nc.vector.memset(xs_bd, 0.0)
            nc.vector.memset(xs_bd[0:64, :, 32:33], 1.0)
            nc.vector.memset(xs_bd[64:128, :, 65:66], 1.0)
            rec6 = p_sm.tile([128, 6], F32, tag="rec")
            nc.vector.reciprocal(rec6[0:64, :], xs_acc[0:64, :, 32])
            nc.vector.reciprocal(rec6[64:128, :], xs_acc[64:128, :, 65])
            nc.vector.tensor_mul(xs_bd[0:64, :, 0:32], xs_acc[0:64, :, 0:32],
                                 rec6[0:64, :].unsqueeze(2).to_broadcast([64, 6, 32]))
            nc.vector.tensor_mul(xs_bd[64:128, :, 33:65], xs_acc[64:128, :, 33:65],
                                 rec6[64:128, :].unsqueeze(2).to_broadcast([64, 6, 32]))
            if not zero_bias:
                nc.vector.tensor_add(xs_bd[0:64, :, 0:32], xs_bd[0:64, :, 0:32],
                                     bvb[0:64])
                nc.vector.tensor_add(xs_bd[64:128, :, 33:65], xs_bd[64:128, :, 33:65],
                                     bvb[64:128])

            # ---- stage 2: branch-A attention + proj ----
            for c in range(NCH):
                for t in range(TPC):
                    s0 = c * CH + t * 128
                    xo = psC.tile([128, 396], F32, tag="pC")
                    for j in range(6):
                        nc.tensor.matmul(xo[:, j * 66:(j + 1) * 66],
                                         lhsT=pa_full[:, j, s0:s0 + 128],
                                         rhs=xs_bd[:, j, :],
                                         start=True, stop=True)
                    xo3 = xo[:].rearrange("p (h d) -> p h d", d=33)
                    rec = p_sm.tile([128, 12], F32, tag="rec12")
                    nc.vector.reciprocal(rec, xo3[:, :, 32])
                    xon = p_xon.tile([128, C], BF16)
                    nc.vector.tensor_mul(xon[:].rearrange("p (h d) -> p h d", d=32),
                                         xo3[:, :, 0:32],
                                         rec[:].unsqueeze(2).to_broadcast([128, 12, 32]))
                    pr = psA.tile([128, C], F32, tag="pA")
                    for f in range(3):
                        tp = psC.tile([128, 128], BF16, tag="pC")
                        nc.tensor.transpose(tp, xon[:, f * 128:(f + 1) * 128], ident)
                        xot = p_xot.tile([128, 128], BF16)
                        nc.vector.tensor_copy(xot, tp)
                        nc.tensor.matmul(pr, lhsT=xot, rhs=w_pr[:, f, :],
                                         start=(f == 0), stop=(f == 2),
                                         skip_group_check=True)
                    o_sb = p_out.tile([128, C], BF16)
                    nc.scalar.copy(o_sb, pr)
                    nc.sync.dma_start(out=out[s0:s0 + 128, :], in_=o_sb)
    if finalize:
        nc.finalize()
    return nc


def _prep_host(inputs):
    f32 = np.float32
    x = np.asarray(inputs["x"], f32)
    attn = np.asarray(inputs["attn"], f32)
    agent = np.asarray(inputs["agent_input"], f32)
    wa = np.asarray(inputs["wa"], f32)
    wb = np.asarray(inputs["wb"], f32)

    perm = np.empty(C2, np.int64)
    sva = np.empty(C2, f32)
    svb = np.empty(C2, f32)
    for h in range(H):
        for br in range(2):
            j0 = h * 64 + br * 32
            perm[j0:j0 + 32] = br * C + h * 32 + np.arange(32)
            sva[j0:j0 + 32] = wa[br] * SCALE
            svb[j0:j0 + 32] = wb[br] * SCALE

    wq_p = np.asarray(inputs["Wq_lf"], f32)[:, perm]
    bq_p = np.asarray(inputs["bq_lf"], f32)[perm]
    wkag_p = np.asarray(inputs["Wk_ag"], f32)[:, perm] * sva[None, :]
    bkag_p = np.asarray(inputs["bk_ag"], f32)[perm] * sva
    wqag_p = np.asarray(inputs["Wq_ag"], f32)[:, perm]
    bqag_p = np.asarray(inputs["bq_ag"], f32)[perm]
    wkhf_p = np.asarray(inputs["Wk_hf"], f32)[:, perm] * svb[None, :]

    zb = all(not np.any(np.asarray(inputs[k]))
             for k in ("bq_lf", "bk_ag", "bq_ag", "bk_hf", "bv_hf", "ba", "bb"))

    shared = {
        "wqT": np.ascontiguousarray(wq_p.T).astype(NPBF16),
        "wkhfT": np.ascontiguousarray(wkhf_p.T).astype(NPBF16),
        "wkag": wkag_p.astype(NPBF16),
        "wqag": wqag_p.astype(NPBF16),
        "wv": np.asarray(inputs["Wv_hf"], f32).astype(NPBF16),
        "wproj": np.asarray(inputs["Wproj"], f32).astype(NPBF16),
    }
    if not zb:
        bv_in = np.asarray(inputs["bv_hf"], f32)
        # bvh[half, j, d]: half 0 = head 2j, half 1 = head 2j+1
        bvh = np.empty((2, NP, D), f32)
        for j in range(NP):
            bvh[0, j, :] = bv_in[(2 * j) * D:(2 * j + 1) * D]
            bvh[1, j, :] = bv_in[(2 * j + 1) * D:(2 * j + 2) * D]
        shared.update({
            "bq": bq_p, "bkag": bkag_p, "bqag": bqag_p,
            "bvh": np.ascontiguousarray(bvh.reshape(-1)),
        })
    xT = np.ascontiguousarray(x.transpose(0, 2, 1)).astype(NPBF16)
    attnT = np.ascontiguousarray(attn.transpose(0, 2, 1)).astype(NPBF16)
    agT = np.ascontiguousarray(agent.transpose(0, 2, 1)).astype(NPBF16)
    in_maps = []
    for b in range(B):
        m = dict(shared)
        m["xT"] = xT[b]
        m["attnT"] = attnT[b]
        m["agT"] = agT[b]
        in_maps.append(m)
    return in_maps, zb


def kernel(**inputs):
    in_maps, zb = _prep_host(inputs)
    key = ("nc", zb)
    if key not in _CACHE:
        _CACHE[key] = _build_bass(zero_bias=zb)
    nc = _CACHE[key]
    res = run_bass_kernel_spmd(nc, in_maps, core_ids=list(range(B)))
    outs = np.stack([np.asarray(res.results[b]["out"], np.float32)
                     for b in range(B)], axis=0)
    if not zb:
        outs = outs + np.asarray(inputs["bproj"], np.float32)[None, None, :]
    return outs


# revision 17
# speedup vs baseline: 2.2343x; 1.4476x over previous
"""Trainium2 Bass kernel for the dual-branch agent-attention module.

Sharding: data-parallel over B=8 (one batch element per NeuronCore).
All transposes and weight permutations are done host-side; on-device
work is a streamed bf16 pipeline.

Math restructuring vs the reference:
  - Effective score weights Weff_A = Wq @ k12bd and Weff_B = Wkhf @ qabd
    (associativity: the big activations never materialize q or kh).
  - Scalar softmax biases ba/bb cancel (softmax shift invariance) and
    are dropped. Branch-A's per-agent bias c_A = k12bd^T @ bq survives
    as the exp's per-partition bias.
  - v bias bv is folded in AFTER the xs softmax-normalize
    (xs_n = xs0/denom + bv), so the per-tile v-bias matmul disappears;
    the softmax denominators come from ones columns memset into the
    v tile.
  - proj bias is added host-side.

Dataflow (per core):
  stage 1 (per 512-col chunk of N):
    B: v = attnT^T@Wv, scores t = attnT^T@Weff_B (wide 512/256 rhs,
       stationary operand shared k-major), exp on ACT straight from
       PSUM, xs accumulated in PSUM across all 32 seq tiles (a single
       K=1 zeroing matmul opens the accumulation region).
    AC: scores s = Weff_A^T@xT per head pair, exp(+c_A) into a
       persistent SBUF pa buffer.
  stage 1.5: xs normalize -> block-diag [xs | 1] tiles.
  stage 2 (per seq tile): x_out = PA^T @ xs_bd with ones-column
    denominators, normalize, PE-transpose, proj, store.
"""

import os
import sys
import numpy as np

for _p in ("/opt/trn_rl_repo", os.path.expanduser("~/.axon_site/_ro/trn_rl_repo")):
    if os.path.isdir(_p) and _p not in sys.path:
        sys.path.insert(0, _p)

import ml_dtypes

import concourse.bass as bass
import concourse.bacc as bacc
import concourse.tile as tile
from concourse import mybir
from concourse.bass_utils import run_bass_kernel_spmd
from concourse.masks import make_identity

BF16 = mybir.dt.bfloat16
F32 = mybir.dt.float32
NPBF16 = ml_dtypes.bfloat16

B, N, NA, H, D = 8, 4096, 64, 12, 32
C = H * D            # 384
C2 = 2 * C           # 768
NP = H // 2          # 6 head pairs
CH = 512             # seq chunk
NCH = N // CH        # 8
TPC = CH // 128      # 4 seq tiles per chunk
SCALE = D ** -0.5

_CACHE = {}


def _build_bass(finalize=True, zero_bias=False):
    nc = bacc.Bacc()

    # ---- DRAM I/O ----
    xT = nc.dram_tensor("xT", [C, N], BF16, kind="ExternalInput")
    attnT = nc.dram_tensor("attnT", [C, N], BF16, kind="ExternalInput")
    agT = nc.dram_tensor("agT", [C, NA], BF16, kind="ExternalInput")
    wqT = nc.dram_tensor("wqT", [C2, C], BF16, kind="ExternalInput")
    wkag = nc.dram_tensor("wkag", [C, C2], BF16, kind="ExternalInput")
    wqag = nc.dram_tensor("wqag", [C, C2], BF16, kind="ExternalInput")
    wkhfT = nc.dram_tensor("wkhfT", [C2, C], BF16, kind="ExternalInput")
    wv = nc.dram_tensor("wv", [C, C], BF16, kind="ExternalInput")
    wproj = nc.dram_tensor("wproj", [C, C], BF16, kind="ExternalInput")
    if not zero_bias:
        bq = nc.dram_tensor("bq", [C2], F32, kind="ExternalInput")
        bkag = nc.dram_tensor("bkag", [C2], F32, kind="ExternalInput")
        bqag = nc.dram_tensor("bqag", [C2], F32, kind="ExternalInput")
        bvh = nc.dram_tensor("bvh", [2 * NP * D], F32, kind="ExternalInput")
    out = nc.dram_tensor("out", [N, C], BF16, kind="ExternalOutput")

    Exp = mybir.ActivationFunctionType.Exp

    with tile.TileContext(nc) as tc:
        with (
            tc.tile_pool(name="const", bufs=1) as const,
            tc.tile_pool(name="vv", bufs=2) as p_v,
            tc.tile_pool(name="pt", bufs=3) as p_pt,
            tc.tile_pool(name="xon", bufs=2) as p_xon,
            tc.tile_pool(name="xot", bufs=3) as p_xot,
            tc.tile_pool(name="osb", bufs=3) as p_out,
            tc.tile_pool(name="sm", bufs=4) as p_sm,
            tc.tile_pool(name="psA", bufs=3, space="PSUM") as psA,
            tc.tile_pool(name="psC", bufs=2, space="PSUM") as psC,
            tc.tile_pool(name="psT", bufs=2, space="PSUM") as psT,
            tc.tile_pool(name="psX", bufs=1, space="PSUM") as psX,
            # psA: 3 banks (ps4/AC-scores/pr), psC: 2 banks (v/ps2/xo),
            # psT: 2 banks (transposes), psX: 1 bank (xs acc) -> 8 total.
        ):
            # ---- constants ----
            w_qT = const.tile([128, 6, C], BF16)
            w_khfT = const.tile([128, 6, C], BF16)
            w_kag = const.tile([128, 3, C2], BF16)
            w_qag = const.tile([128, 3, C2], BF16)
            w_v = const.tile([128, 3, C], BF16)
            w_pr = const.tile([128, 3, C], BF16)
            for dst, src in ((w_kag, wkag), (w_qag, wqag), (w_qT, wqT),
                             (w_khfT, wkhfT), (w_v, wv), (w_pr, wproj)):
                nc.sync.dma_start(out=dst, in_=src.rearrange("(k p) m -> p k m", p=128))
            ag_t = const.tile([128, 3, NA], BF16)
            nc.gpsimd.dma_start(out=ag_t, in_=agT.rearrange("(k p) m -> p k m", p=128))
            if not zero_bias:
                b_q = const.tile([128, 6], F32)
                b_kag = const.tile([128, 6], F32)
                b_qag = const.tile([128, 6], F32)
                for dst, src in ((b_q, bq), (b_kag, bkag), (b_qag, bqag)):
                    nc.gpsimd.dma_start(out=dst, in_=src.rearrange("(j p) -> p j", p=128))
                bvb = const.tile([128, NP, D], F32)
                nc.gpsimd.dma_start(
                    out=bvb[0:64],
                    in_=bass.AP(tensor=bvh[:].tensor, offset=0,
                                ap=[[0, 64], [1, NP * D]]))
                nc.gpsimd.dma_start(
                    out=bvb[64:128],
                    in_=bass.AP(tensor=bvh[:].tensor, offset=NP * D,
                                ap=[[0, 64], [1, NP * D]]))
            ident = const.tile([128, 128], BF16)
            make_identity(nc, ident)
            zrow = const.tile([1, 396], BF16)
            nc.vector.memset(zrow, 0.0)

            # full activations resident in SBUF
            at_full = const.tile([128, 3, N], BF16)
            xt_full = const.tile([128, 3, N], BF16)
            for c in range(NCH):
                nc.sync.dma_start(
                    out=at_full[:, :, c * CH:(c + 1) * CH],
                    in_=attnT.rearrange("(k p) s -> p k s", p=128)[:, :, c * CH:(c + 1) * CH])
                nc.sync.dma_start(
                    out=xt_full[:, :, c * CH:(c + 1) * CH],
                    in_=xT.rearrange("(k p) s -> p k s", p=128)[:, :, c * CH:(c + 1) * CH])
            pa_full = const.tile([128, 6, N], BF16)

            # Pre-touch DMA-loaded bias constants with tiny reads.
            if not zero_bias:
                touch = const.tile([128, 16], F32)
                for i, t_ap in enumerate((b_q[:, 0:1], b_kag[:, 0:1],
                                          b_qag[:, 0:1], bvb[:, 0:1, 0])):
                    nc.vector.tensor_copy(touch[:, i:i + 1], t_ap)

            # ---- prep: k_ag / qa projections -> block-diag tiles ----
            kag_sb = const.tile([128, 6, NA], BF16)
            qa_sb = const.tile([128, 6, NA], BF16)
            for w_t, b_t, dst in ((w_kag, "bkag", kag_sb), (w_qag, "bqag", qa_sb)):
                for j in range(6):
                    ps = psA.tile([128, NA], F32, tag="pA")
                    for k in range(3):
                        nc.tensor.matmul(ps, lhsT=w_t[:, k, j * 128:(j + 1) * 128],
                                         rhs=ag_t[:, k, :], start=(k == 0), stop=(k == 2))
                    if zero_bias:
                        nc.vector.tensor_copy(dst[:, j, :], ps)
                    else:
                        bt = b_kag if b_t == "bkag" else b_qag
                        nc.vector.tensor_add(dst[:, j, :], ps,
                                             bt[:, j:j + 1].to_broadcast([128, NA]))
            k12bd = const.tile([128, 6, 128], BF16)
            qabd = const.tile([128, 6, 128], BF16)
            for src, dst in ((kag_sb, k12bd), (qa_sb, qabd)):
                nc.vector.memset(dst, 0.0)
                for j in range(6):
                    nc.vector.tensor_copy(dst[0:64, j, 0:64], src[0:64, j, :])
                    nc.vector.tensor_copy(dst[64:128, j, 64:128], src[64:128, j, :])

            # ---- prep: effective score weights + branch-A exp bias ----
            # weff_b first: stage-1 B work only needs weff_b, so the PE can
            # enter the main loop while weff_a is still being produced.
            weff_a = const.tile([128, 3, C2], BF16)
            weff_b = const.tile([128, 3, C2], BF16)
            for j in range(6):
                for k in range(3):
                    ps2 = psA.tile([128, 128], F32, tag="pA")
                    nc.tensor.matmul(ps2, lhsT=w_khfT[:, j, k * 128:(k + 1) * 128],
                                     rhs=qabd[:, j, :], start=True, stop=True)
                    nc.scalar.copy(weff_b[:, k, j * 128:(j + 1) * 128], ps2)
            for j in range(6):
                for k in range(3):
                    ps = psA.tile([128, 128], F32, tag="pA")
                    nc.tensor.matmul(ps, lhsT=w_qT[:, j, k * 128:(k + 1) * 128],
                                     rhs=k12bd[:, j, :], start=True, stop=True)
                    nc.vector.tensor_copy(weff_a[:, k, j * 128:(j + 1) * 128], ps)
            cba = None
            if not zero_bias:
                b_q_bf = const.tile([128, 6], BF16)
                nc.vector.tensor_copy(b_q_bf, b_q)
                cba = const.tile([128, 6], F32)
                for j in range(6):
                    ps = psA.tile([128, 1], F32, tag="pA")
                    nc.tensor.matmul(ps, lhsT=k12bd[:, j, :], rhs=b_q_bf[:, j:j + 1],
                                     start=True, stop=True)
                    nc.vector.tensor_copy(cba[:, j:j + 1], ps)

            # ---- xs accumulator: open the PSUM region with a zero matmul ----
            xs_acc = psX.tile([128, 6, 66], F32)
            nc.tensor.matmul(xs_acc[:, :, :], lhsT=zrow[:, 0:128], rhs=zrow[:, 0:396],
                             start=True, stop=False, skip_group_check=True)

            # ---- stage 1: values + branch-B attention (xs in PSUM) ----
            pending_xs = None
            for c in range(NCH):
                v_t = p_v.tile([128, TPC, H, 33], BF16)
                nc.vector.memset(v_t[:, :, :, 32], 1.0)
                for t in range(TPC):
                    s0 = c * CH + t * 128
                    psv = psC.tile([128, C], F32, tag="pC")
                    ps4 = psA.tile([128, 512], F32, tag="pA")
                    ps2 = psC.tile([128, 256], F32, tag="pC")
                    for k in range(3):
                        at_k = at_full[:, k, s0:s0 + 128]
                        nc.tensor.matmul(psv, lhsT=at_k, rhs=w_v[:, k, :],
                                         start=(k == 0), stop=(k == 2))
                        nc.tensor.matmul(ps4, lhsT=at_k, rhs=weff_b[:, k, 0:512],
                                         start=(k == 0), stop=(k == 2))
                        nc.tensor.matmul(ps2, lhsT=at_k, rhs=weff_b[:, k, 512:768],
                                         start=(k == 0), stop=(k == 2))
                    pt = p_pt.tile([128, 768], BF16)
                    nc.scalar.activation(pt[:, 0:512], ps4, Exp)
                    nc.scalar.activation(pt[:, 512:768], ps2, Exp)
                    nc.vector.tensor_copy(
                        v_t[:, t, :, 0:32],
                        psv[:].rearrange("p (h d) -> p h d", d=32))
                    if pending_xs is not None:
                        pending_xs()
                    last = (c == NCH - 1 and t == TPC - 1)

                    def make_xs(pt=pt, v_t=v_t, t=t, last=last):
                        def emit():
                            for j in range(6):
                                nc.tensor.matmul(
                                    xs_acc[:, j, :], lhsT=pt[:, j * 128:(j + 1) * 128],
                                    rhs=v_t[:, t, 2 * j:2 * j + 2, :],
                                    start=False, stop=(last and j == 5),
                                    skip_group_check=True)
                        return emit
                    pending_xs = make_xs()
            pending_xs()

            # ---- stage 1.5: xs normalize -> block-diag [xs | 1] tiles ----
            xs_bd = const.tile([128, 6, 66], BF16)
            nc.vector.memset(xs_bd, 0.0)
            nc.vector.memset(xs_bd[0:64, :, 32:33], 1.0)
            nc.vector.memset(xs_bd[64:128, :, 65:66], 1.0)
            rec6 = p_sm.tile([128, 6], F32, tag="rec")
            nc.vector.reciprocal(rec6[0:64, :], xs_acc[0:64, :, 32])
            nc.vector.reciprocal(rec6[64:128, :], xs_acc[64:128, :, 65])
            nc.vector.tensor_mul(xs_bd[0:64, :, 0:32], xs_acc[0:64, :, 0:32],
                                 rec6[0:64, :].unsqueeze(2).to_broadcast([64, 6, 32]))
            nc.vector.tensor_mul(xs_bd[64:128, :, 33:65], xs_acc[64:128, :, 33:65],
                                 rec6[64:128, :].unsqueeze(2).to_broadcast([64, 6, 32]))
            if not zero_bias:
                nc.vector.tensor_add(xs_bd[0:64, :, 0:32], xs_bd[0:64, :, 0:32],
                                     bvb[0:64])
                nc.vector.tensor_add(xs_bd[64:128, :, 33:65], xs_bd[64:128, :, 33:65],
                                     bvb[64:128])

            # ---- stage 2: branch-A scores + attention + proj ----
            # Per chunk: branch-A score matmuls + exp into pa_full, then the
            # previous chunk's x_out/proj tiles. The long per-tile serial
            # chains (normalize -> transpose -> proj) hide under the dense
            # score streams of the next chunk.
            def stage2_tiles(c):
                for t in range(TPC):
                    s0 = c * CH + t * 128
                    xo = psC.tile([128, 396], F32, tag="pC")
                    for j in range(6):
                        nc.tensor.matmul(xo[:, j * 66:(j + 1) * 66],
                                         lhsT=pa_full[:, j, s0:s0 + 128],
                                         rhs=xs_bd[:, j, :],
                                         start=True, stop=True)
                    xo3 = xo[:].rearrange("p (h d) -> p h d", d=33)
                    rec = p_sm.tile([128, 12], F32, tag="rec12")
                    nc.vector.reciprocal(rec, xo3[:, :, 32])
                    xon = p_xon.tile([128, C], BF16)
                    nc.vector.tensor_mul(xon[:].rearrange("p (h d) -> p h d", d=32),
                                         xo3[:, :, 0:32],
                                         rec[:].unsqueeze(2).to_broadcast([128, 12, 32]))
                    pr = psA.tile([128, C], F32, tag="pA")
                    for f in range(3):
                        tp = psT.tile([128, 128], BF16)
                        nc.tensor.transpose(tp, xon[:, f * 128:(f + 1) * 128], ident)
                        xot = p_xot.tile([128, 128], BF16)
                        nc.vector.tensor_copy(xot, tp)
                        nc.tensor.matmul(pr, lhsT=xot, rhs=w_pr[:, f, :],
                                         start=(f == 0), stop=(f == 2),
                                         skip_group_check=True)
                    o_sb = p_out.tile([128, C], BF16)
                    if t % 2 == 0:
                        nc.scalar.copy(o_sb, pr)
                    else:
                        nc.vector.tensor_copy(o_sb, pr)
                    nc.sync.dma_start(out=out[s0:s0 + 128, :], in_=o_sb)

            for c in range(NCH):
                for j in range(6):
                    ps = psA.tile([128, CH], F32, tag="pA")
                    for k in range(3):
                        nc.tensor.matmul(ps, lhsT=weff_a[:, k, j * 128:(j + 1) * 128],
                                         rhs=xt_full[:, k, c * CH:(c + 1) * CH],
                                         start=(k == 0), stop=(k == 2))
                    nc.scalar.activation(
                        pa_full[:, j, c * CH:(c + 1) * CH], ps, Exp,
                        bias=(0.0 if zero_bias else cba[:, j:j + 1]))
                if c > 0:
                    stage2_tiles(c - 1)
            stage2_tiles(NCH - 1)
    if finalize:
        nc.finalize()
    return nc


def _prep_host(inputs):
    f32 = np.float32
    x = np.asarray(inputs["x"], f32)
    attn = np.asarray(inputs["attn"], f32)
    agent = np.asarray(inputs["agent_input"], f32)
    wa = np.asarray(inputs["wa"], f32)
    wb = np.asarray(inputs["wb"], f32)

    perm = np.empty(C2, np.int64)
    sva = np.empty(C2, f32)
    svb = np.empty(C2, f32)
    for h in range(H):
        for br in range(2):
            j0 = h * 64 + br * 32
            perm[j0:j0 + 32] = br * C + h * 32 + np.arange(32)
            sva[j0:j0 + 32] = wa[br] * SCALE
            svb[j0:j0 + 32] = wb[br] * SCALE

    wq_p = np.asarray(inputs["Wq_lf"], f32)[:, perm]
    bq_p = np.asarray(inputs["bq_lf"], f32)[perm]
    wkag_p = np.asarray(inputs["Wk_ag"], f32)[:, perm] * sva[None, :]
    bkag_p = np.asarray(inputs["bk_ag"], f32)[perm] * sva
    wqag_p = np.asarray(inputs["Wq_ag"], f32)[:, perm]
    bqag_p = np.asarray(inputs["bq_ag"], f32)[perm]
    wkhf_p = np.asarray(inputs["Wk_hf"], f32)[:, perm] * svb[None, :]

    zb = all(not np.any(np.asarray(inputs[k]))
             for k in ("bq_lf", "bk_ag", "bq_ag", "bk_hf", "bv_hf", "ba", "bb"))

    shared = {
        "wqT": np.ascontiguousarray(wq_p.T).astype(NPBF16),
        "wkhfT": np.ascontiguousarray(wkhf_p.T).astype(NPBF16),
        "wkag": wkag_p.astype(NPBF16),
        "wqag": wqag_p.astype(NPBF16),
        "wv": np.asarray(inputs["Wv_hf"], f32).astype(NPBF16),
        "wproj": np.asarray(inputs["Wproj"], f32).astype(NPBF16),
    }
    if not zb:
        bv_in = np.asarray(inputs["bv_hf"], f32)
        # bvh[half, j, d]: half 0 = head 2j, half 1 = head 2j+1
        bvh = np.empty((2, NP, D), f32)
        for j in range(NP):
            bvh[0, j, :] = bv_in[(2 * j) * D:(2 * j + 1) * D]
            bvh[1, j, :] = bv_in[(2 * j + 1) * D:(2 * j + 2) * D]
        shared.update({
            "bq": bq_p, "bkag": bkag_p, "bqag": bqag_p,
            "bvh": np.ascontiguousarray(bvh.reshape(-1)),
        })
    xT = np.ascontiguousarray(x.transpose(0, 2, 1)).astype(NPBF16)
    attnT = np.ascontiguousarray(attn.transpose(0, 2, 1)).astype(NPBF16)
    agT = np.ascontiguousarray(agent.transpose(0, 2, 1)).astype(NPBF16)
    in_maps = []
    for b in range(B):
        m = dict(shared)
        m["xT"] = xT[b]
        m["attnT"] = attnT[b]
        m["agT"] = agT[b]
        in_maps.append(m)
    return in_maps, zb


def kernel(**inputs):
    in_maps, zb = _prep_host(inputs)
    key = ("nc", zb)
    if key not in _CACHE:
        _CACHE[key] = _build_bass(zero_bias=zb)
    nc = _CACHE[key]
    res = run_bass_kernel_spmd(nc, in_maps, core_ids=list(range(B)))
    outs = np.stack([np.asarray(res.results[b]["out"], np.float32)
                     for b in range(B)], axis=0)
    if not zb:
        outs = outs + np.asarray(inputs["bproj"], np.float32)[None, None, :]
    return outs


# revision 27
# speedup vs baseline: 2.2846x; 1.0225x over previous
"""Trainium2 Bass kernel for the dual-branch agent-attention module.

Sharding: data-parallel over B=8 (one batch element per NeuronCore).
All transposes and weight permutations are done host-side; on-device
work is a streamed bf16 pipeline.

Math restructuring vs the reference:
  - Effective score weights Weff_A = Wq @ k12bd and Weff_B = Wkhf @ qabd
    (associativity: the big activations never materialize q or kh).
  - Scalar softmax biases ba/bb cancel (softmax shift invariance) and
    are dropped. Branch-A's per-agent bias c_A = k12bd^T @ bq survives
    as the exp's per-partition bias.
  - v bias bv is folded in AFTER the xs softmax-normalize
    (xs_n = xs0/denom + bv), so the per-tile v-bias matmul disappears;
    the softmax denominators come from ones columns memset into the
    v tile.
  - proj bias is added host-side.

Dataflow (per core):
  stage 1 (per 512-col chunk of N):
    B: v = attnT^T@Wv, scores t = attnT^T@Weff_B (wide 512/256 rhs,
       stationary operand shared k-major), exp on ACT straight from
       PSUM, xs accumulated in PSUM across all 32 seq tiles (a single
       K=1 zeroing matmul opens the accumulation region).
    AC: scores s = Weff_A^T@xT per head pair, exp(+c_A) into a
       persistent SBUF pa buffer.
  stage 1.5: xs normalize -> block-diag [xs | 1] tiles.
  stage 2 (per seq tile): x_out = PA^T @ xs_bd with ones-column
    denominators, normalize, PE-transpose, proj, store.
"""

import os
import sys
import numpy as np

for _p in ("/opt/trn_rl_repo", os.path.expanduser("~/.axon_site/_ro/trn_rl_repo")):
    if os.path.isdir(_p) and _p not in sys.path:
        sys.path.insert(0, _p)

import ml_dtypes

import concourse.bass as bass
import concourse.bacc as bacc
import concourse.tile as tile
from concourse import mybir
from concourse.bass_utils import run_bass_kernel_spmd
from concourse.masks import make_identity

BF16 = mybir.dt.bfloat16
F32 = mybir.dt.float32
NPBF16 = ml_dtypes.bfloat16

B, N, NA, H, D = 8, 4096, 64, 12, 32
C = H * D            # 384
C2 = 2 * C           # 768
NP = H // 2          # 6 head pairs
CH = 512             # seq chunk
NCH = N // CH        # 8
TPC = CH // 128      # 4 seq tiles per chunk
SCALE = D ** -0.5

_CACHE = {}


def _build_bass(finalize=True, zero_bias=False):
    nc = bacc.Bacc()

    # ---- DRAM I/O ----
    xT = nc.dram_tensor("xT", [C, N], BF16, kind="ExternalInput")
    attnT = nc.dram_tensor("attnT", [C, N], BF16, kind="ExternalInput")
    agT = nc.dram_tensor("agT", [C, NA], BF16, kind="ExternalInput")
    wqT = nc.dram_tensor("wqT", [C2, C], BF16, kind="ExternalInput")
    wkag = nc.dram_tensor("wkag", [C, C2], BF16, kind="ExternalInput")
    wqag = nc.dram_tensor("wqag", [C, C2], BF16, kind="ExternalInput")
    wkhfT = nc.dram_tensor("wkhfT", [C2, C], BF16, kind="ExternalInput")
    wv = nc.dram_tensor("wv", [C, C], BF16, kind="ExternalInput")
    wproj = nc.dram_tensor("wproj", [C, C], BF16, kind="ExternalInput")
    if not zero_bias:
        bq = nc.dram_tensor("bq", [C2], F32, kind="ExternalInput")
        bkag = nc.dram_tensor("bkag", [C2], F32, kind="ExternalInput")
        bqag = nc.dram_tensor("bqag", [C2], F32, kind="ExternalInput")
        bvh = nc.dram_tensor("bvh", [2 * NP * D], F32, kind="ExternalInput")
    out = nc.dram_tensor("out", [N, C], BF16, kind="ExternalOutput")

    Exp = mybir.ActivationFunctionType.Exp

    with tile.TileContext(nc) as tc:
        with (
            tc.tile_pool(name="const", bufs=1) as const,
            tc.tile_pool(name="vv", bufs=2) as p_v,
            tc.tile_pool(name="pt", bufs=3) as p_pt,
            tc.tile_pool(name="xon", bufs=2) as p_xon,
            tc.tile_pool(name="xot", bufs=3) as p_xot,
            tc.tile_pool(name="osb", bufs=3) as p_out,
            tc.tile_pool(name="sm", bufs=4) as p_sm,
            tc.tile_pool(name="psA", bufs=3, space="PSUM") as psA,
            tc.tile_pool(name="psC", bufs=2, space="PSUM") as psC,
            tc.tile_pool(name="psT", bufs=2, space="PSUM") as psT,
            tc.tile_pool(name="psX", bufs=1, space="PSUM") as psX,
            # psA: 3 banks (ps4/AC-scores/pr), psC: 2 banks (v/ps2/xo),
            # psT: 2 banks (transposes), psX: 1 bank (xs acc) -> 8 total.
        ):
            # ---- constants ----
            w_qT = const.tile([128, 6, C], BF16)
            w_khfT = const.tile([128, 6, C], BF16)
            w_kag = const.tile([128, 3, C2], BF16)
            w_qag = const.tile([128, 3, C2], BF16)
            w_v = const.tile([128, 3, C], BF16)
            w_pr = const.tile([128, 3, C], BF16)
            for dst, src in ((w_kag, wkag), (w_qag, wqag), (w_v, wv)):
                nc.sync.dma_start(out=dst, in_=src.rearrange("(k p) m -> p k m", p=128))
            ag_t = const.tile([128, 3, NA], BF16)
            nc.gpsimd.dma_start(out=ag_t, in_=agT.rearrange("(k p) m -> p k m", p=128))
            if not zero_bias:
                b_q = const.tile([128, 6], F32)
                b_kag = const.tile([128, 6], F32)
                b_qag = const.tile([128, 6], F32)
                for dst, src in ((b_q, bq), (b_kag, bkag), (b_qag, bqag)):
                    nc.gpsimd.dma_start(out=dst, in_=src.rearrange("(j p) -> p j", p=128))
                bvb = const.tile([128, NP, D], F32)
                nc.gpsimd.dma_start(
                    out=bvb[0:64],
                    in_=bass.AP(tensor=bvh[:].tensor, offset=0,
                                ap=[[0, 64], [1, NP * D]]))
                nc.gpsimd.dma_start(
                    out=bvb[64:128],
                    in_=bass.AP(tensor=bvh[:].tensor, offset=NP * D,
                                ap=[[0, 64], [1, NP * D]]))
            ident = const.tile([128, 128], BF16)
            make_identity(nc, ident)
            zrow = const.tile([1, 396], BF16)
            nc.vector.memset(zrow, 0.0)

            # full activations resident in SBUF (chunk 0 of attnT first so the
            # chunk-0 v matmuls can start during prep)
            at_full = const.tile([128, 3, N], BF16)
            xt_full = const.tile([128, 3, N], BF16)
            at_r = attnT.rearrange("(k p) s -> p k s", p=128)
            xt_r = xT.rearrange("(k p) s -> p k s", p=128)
            nc.sync.dma_start(out=at_full[:, :, 0:CH], in_=at_r[:, :, 0:CH])
            for dst, src in ((w_qT, wqT), (w_khfT, wkhfT), (w_pr, wproj)):
                nc.sync.dma_start(out=dst, in_=src.rearrange("(k p) m -> p k m", p=128))
            for c in range(NCH):
                if c > 0:
                    nc.sync.dma_start(
                        out=at_full[:, :, c * CH:(c + 1) * CH],
                        in_=at_r[:, :, c * CH:(c + 1) * CH])
                nc.sync.dma_start(
                    out=xt_full[:, :, c * CH:(c + 1) * CH],
                    in_=xt_r[:, :, c * CH:(c + 1) * CH])
            pa_full = const.tile([128, 6, N], BF16)

            # Pre-touch DMA-loaded bias constants with tiny reads.
            if not zero_bias:
                touch = const.tile([128, 16], F32)
                for i, t_ap in enumerate((b_q[:, 0:1], b_kag[:, 0:1],
                                          b_qag[:, 0:1], bvb[:, 0:1, 0])):
                    nc.vector.tensor_copy(touch[:, i:i + 1], t_ap)

            # ---- prep: k_ag / qa projections -> block-diag tiles ----
            kag_sb = const.tile([128, 6, NA], BF16)
            qa_sb = const.tile([128, 6, NA], BF16)
            for w_t, b_t, dst in ((w_qag, "bqag", qa_sb), (w_kag, "bkag", kag_sb)):
                for j in range(6):
                    ps = psA.tile([128, NA], F32, tag="pA")
                    for k in range(3):
                        nc.tensor.matmul(ps, lhsT=w_t[:, k, j * 128:(j + 1) * 128],
                                         rhs=ag_t[:, k, :], start=(k == 0), stop=(k == 2))
                    if zero_bias:
                        if b_t == "bqag":
                            nc.scalar.copy(dst[:, j, :], ps)
                        else:
                            nc.vector.tensor_copy(dst[:, j, :], ps)
                    else:
                        bt = b_kag if b_t == "bkag" else b_qag
                        nc.vector.tensor_add(dst[:, j, :], ps,
                                             bt[:, j:j + 1].to_broadcast([128, NA]))
            k12bd = const.tile([128, 6, 128], BF16)
            qabd = const.tile([128, 6, 128], BF16)
            nc.gpsimd.memset(qabd, 0.0)
            nc.vector.memset(k12bd, 0.0)
            for j in range(6):
                nc.scalar.copy(qabd[0:64, j, 0:64], qa_sb[0:64, j, :])
                nc.scalar.copy(qabd[64:128, j, 64:128], qa_sb[64:128, j, :])
                nc.vector.tensor_copy(k12bd[0:64, j, 0:64], kag_sb[0:64, j, :])
                nc.vector.tensor_copy(k12bd[64:128, j, 64:128], kag_sb[64:128, j, :])

            # ---- prep: effective score weights + branch-A exp bias ----
            # weff_b first: stage-1 B work only needs weff_b, so the PE can
            # enter the main loop while weff_a is still being produced.
            weff_a = const.tile([128, 3, C2], BF16)
            weff_b = const.tile([128, 3, C2], BF16)
            for j in range(6):
                for k in range(3):
                    ps2 = psA.tile([128, 128], F32, tag="pA")
                    nc.tensor.matmul(ps2, lhsT=w_khfT[:, j, k * 128:(k + 1) * 128],
                                     rhs=qabd[:, j, :], start=True, stop=True)
                    nc.scalar.copy(weff_b[:, k, j * 128:(j + 1) * 128], ps2)
            for j in range(6):
                for k in range(3):
                    ps = psA.tile([128, 128], F32, tag="pA")
                    nc.tensor.matmul(ps, lhsT=w_qT[:, j, k * 128:(k + 1) * 128],
                                     rhs=k12bd[:, j, :], start=True, stop=True)
                    nc.vector.tensor_copy(weff_a[:, k, j * 128:(j + 1) * 128], ps)
            cba = None
            if not zero_bias:
                b_q_bf = const.tile([128, 6], BF16)
                nc.vector.tensor_copy(b_q_bf, b_q)
                cba = const.tile([128, 6], F32)
                for j in range(6):
                    ps = psA.tile([128, 1], F32, tag="pA")
                    nc.tensor.matmul(ps, lhsT=k12bd[:, j, :], rhs=b_q_bf[:, j:j + 1],
                                     start=True, stop=True)
                    nc.vector.tensor_copy(cba[:, j:j + 1], ps)

            # ---- xs accumulator: open the PSUM region with a zero matmul ----
            xs_acc = psX.tile([128, 6, 66], F32)
            nc.tensor.matmul(xs_acc[:, :, :], lhsT=zrow[:, 0:128], rhs=zrow[:, 0:396],
                             start=True, stop=False, skip_group_check=True)

            # ---- stage 1: values + branch-B attention (xs in PSUM) ----
            # v matmuls for chunk c+1 are emitted during chunk c (chunk 0's
            # before the weff prep above has drained), so PE always has ready
            # work and the v copies pace evenly on DVE.
            v_tiles = {}

            def emit_v(c):
                v_t = p_v.tile([128, TPC, H, 33], BF16)
                nc.vector.memset(v_t[:, :, :, 32], 1.0)
                for t in range(TPC):
                    s0 = c * CH + t * 128
                    psv = psT.tile([128, C], F32, tag="pT")
                    for k in range(3):
                        nc.tensor.matmul(psv, lhsT=at_full[:, k, s0:s0 + 128],
                                         rhs=w_v[:, k, :],
                                         start=(k == 0), stop=(k == 2))
                    nc.vector.tensor_copy(
                        v_t[:, t, :, 0:32],
                        psv[:].rearrange("p (h d) -> p h d", d=32))
                v_tiles[c] = v_t

            emit_v(0)
            pending_xs = None
            for c in range(NCH):
                v_t = v_tiles.pop(c)
                for t in range(TPC):
                    s0 = c * CH + t * 128
                    ps4 = psA.tile([128, 512], F32, tag="pA")
                    ps2 = psC.tile([128, 256], F32, tag="pC")
                    for k in range(3):
                        at_k = at_full[:, k, s0:s0 + 128]
                        nc.tensor.matmul(ps4, lhsT=at_k, rhs=weff_b[:, k, 0:512],
                                         start=(k == 0), stop=(k == 2))
                        nc.tensor.matmul(ps2, lhsT=at_k, rhs=weff_b[:, k, 512:768],
                                         start=(k == 0), stop=(k == 2))
                    pt = p_pt.tile([128, 768], BF16)
                    nc.scalar.activation(pt[:, 0:512], ps4, Exp)
                    nc.scalar.activation(pt[:, 512:768], ps2, Exp)
                    if pending_xs is not None:
                        pending_xs()
                    if t == 1 and c + 1 < NCH:
                        emit_v(c + 1)
                    last = (c == NCH - 1 and t == TPC - 1)

                    def make_xs(pt=pt, v_t=v_t, t=t, last=last):
                        def emit():
                            for j in range(6):
                                nc.tensor.matmul(
                                    xs_acc[:, j, :], lhsT=pt[:, j * 128:(j + 1) * 128],
                                    rhs=v_t[:, t, 2 * j:2 * j + 2, :],
                                    start=False, stop=(last and j == 5),
                                    skip_group_check=True)
                        return emit
                    pending_xs = make_xs()
            pending_xs()

            # ---- stage 1.5: xs normalize -> block-diag [xs | 1] tiles ----
            xs_bd = const.tile([128, 6, 66], BF16)
            nc.vector.memset(xs_bd, 0.0)
            nc.vector.memset(xs_bd[0:64, :, 32:33], 1.0)
            nc.vector.memset(xs_bd[64:128, :, 65:66], 1.0)
            rec6 = p_sm.tile([128, 6], F32, tag="rec")
            nc.vector.reciprocal(rec6[0:64, :], xs_acc[0:64, :, 32])
            nc.vector.reciprocal(rec6[64:128, :], xs_acc[64:128, :, 65])
            nc.vector.tensor_mul(xs_bd[0:64, :, 0:32], xs_acc[0:64, :, 0:32],
                                 rec6[0:64, :].unsqueeze(2).to_broadcast([64, 6, 32]))
            nc.vector.tensor_mul(xs_bd[64:128, :, 33:65], xs_acc[64:128, :, 33:65],
                                 rec6[64:128, :].unsqueeze(2).to_broadcast([64, 6, 32]))
            if not zero_bias:
                nc.vector.tensor_add(xs_bd[0:64, :, 0:32], xs_bd[0:64, :, 0:32],
                                     bvb[0:64])
                nc.vector.tensor_add(xs_bd[64:128, :, 33:65], xs_bd[64:128, :, 33:65],
                                     bvb[64:128])

            # ---- stage 2: branch-A scores + attention + proj ----
            # Per chunk: branch-A score matmuls + exp into pa_full, then the
            # previous chunk's x_out/proj tiles. The long per-tile serial
            # chains (normalize -> transpose -> proj) hide under the dense
            # score streams of the next chunk.
            def stage2_tiles(c, ts):
                for t in ts:
                    s0 = c * CH + t * 128
                    xo = psC.tile([128, 396], F32, tag="pC")
                    for j in range(6):
                        nc.tensor.matmul(xo[:, j * 66:(j + 1) * 66],
                                         lhsT=pa_full[:, j, s0:s0 + 128],
                                         rhs=xs_bd[:, j, :],
                                         start=True, stop=True)
                    xo3 = xo[:].rearrange("p (h d) -> p h d", d=33)
                    rec = p_sm.tile([128, 12], F32, tag="rec12")
                    nc.vector.reciprocal(rec, xo3[:, :, 32])
                    xon = p_xon.tile([128, C], BF16)
                    nc.vector.tensor_mul(xon[:].rearrange("p (h d) -> p h d", d=32),
                                         xo3[:, :, 0:32],
                                         rec[:].unsqueeze(2).to_broadcast([128, 12, 32]))
                    pr = psA.tile([128, C], F32, tag="pA")
                    for f in range(3):
                        tp = psT.tile([128, 128], BF16, tag="pT")
                        nc.tensor.transpose(tp, xon[:, f * 128:(f + 1) * 128], ident)
                        xot = p_xot.tile([128, 128], BF16)
                        nc.vector.tensor_copy(xot, tp)
                        nc.tensor.matmul(pr, lhsT=xot, rhs=w_pr[:, f, :],
                                         start=(f == 0), stop=(f == 2),
                                         skip_group_check=True)
                    o_sb = p_out.tile([128, C], BF16)
                    if t % 2 == 0:
                        nc.scalar.copy(o_sb, pr)
                    else:
                        nc.vector.tensor_copy(o_sb, pr)
                    nc.sync.dma_start(out=out[s0:s0 + 128, :], in_=o_sb)

            for c in range(NCH):
                for j in range(6):
                    ps = psA.tile([128, CH], F32, tag="pA")
                    for k in range(3):
                        nc.tensor.matmul(ps, lhsT=weff_a[:, k, j * 128:(j + 1) * 128],
                                         rhs=xt_full[:, k, c * CH:(c + 1) * CH],
                                         start=(k == 0), stop=(k == 2))
                    nc.scalar.activation(
                        pa_full[:, j, c * CH:(c + 1) * CH], ps, Exp,
                        bias=(0.0 if zero_bias else cba[:, j:j + 1]))
                    if c > 0 and j == 2:
                        stage2_tiles(c - 1, (0, 1))
                if c > 0:
                    stage2_tiles(c - 1, (2, 3))
            stage2_tiles(NCH - 1, (0, 1, 2, 3))
    if finalize:
        nc.finalize()
    return nc


def _prep_host(inputs):
    f32 = np.float32
    x = np.asarray(inputs["x"], f32)
    attn = np.asarray(inputs["attn"], f32)
    agent = np.asarray(inputs["agent_input"], f32)
    wa = np.asarray(inputs["wa"], f32)
    wb = np.asarray(inputs["wb"], f32)

    perm = np.empty(C2, np.int64)
    sva = np.empty(C2, f32)
    svb = np.empty(C2, f32)
    for h in range(H):
        for br in range(2):
            j0 = h * 64 + br * 32
            perm[j0:j0 + 32] = br * C + h * 32 + np.arange(32)
            sva[j0:j0 + 32] = wa[br] * SCALE
            svb[j0:j0 + 32] = wb[br] * SCALE

    wq_p = np.asarray(inputs["Wq_lf"], f32)[:, perm]
    bq_p = np.asarray(inputs["bq_lf"], f32)[perm]
    wkag_p = np.asarray(inputs["Wk_ag"], f32)[:, perm] * sva[None, :]
    bkag_p = np.asarray(inputs["bk_ag"], f32)[perm] * sva
    wqag_p = np.asarray(inputs["Wq_ag"], f32)[:, perm]
    bqag_p = np.asarray(inputs["bq_ag"], f32)[perm]
    wkhf_p = np.asarray(inputs["Wk_hf"], f32)[:, perm] * svb[None, :]

    zb = all(not np.any(np.asarray(inputs[k]))
             for k in ("bq_lf", "bk_ag", "bq_ag", "bk_hf", "bv_hf", "ba", "bb"))

    shared = {
        "wqT": np.ascontiguousarray(wq_p.T).astype(NPBF16),
        "wkhfT": np.ascontiguousarray(wkhf_p.T).astype(NPBF16),
        "wkag": wkag_p.astype(NPBF16),
        "wqag": wqag_p.astype(NPBF16),
        "wv": np.asarray(inputs["Wv_hf"], f32).astype(NPBF16),
        "wproj": np.asarray(inputs["Wproj"], f32).astype(NPBF16),
    }
    if not zb:
        bv_in = np.asarray(inputs["bv_hf"], f32)
        # bvh[half, j, d]: half 0 = head 2j, half 1 = head 2j+1
        bvh = np.empty((2, NP, D), f32)
        for j in range(NP):
            bvh[0, j, :] = bv_in[(2 * j) * D:(2 * j + 1) * D]
            bvh[1, j, :] = bv_in[(2 * j + 1) * D:(2 * j + 2) * D]
        shared.update({
            "bq": bq_p, "bkag": bkag_p, "bqag": bqag_p,
            "bvh": np.ascontiguousarray(bvh.reshape(-1)),
        })
    xT = np.ascontiguousarray(x.transpose(0, 2, 1)).astype(NPBF16)
    attnT = np.ascontiguousarray(attn.transpose(0, 2, 1)).astype(NPBF16)
    agT = np.ascontiguousarray(agent.transpose(0, 2, 1)).astype(NPBF16)
    in_maps = []
    for b in range(B):
        m = dict(shared)
        m["xT"] = xT[b]
        m["attnT"] = attnT[b]
        m["agT"] = agT[b]
        in_maps.append(m)
    return in_maps, zb


def kernel(**inputs):
    in_maps, zb = _prep_host(inputs)
    key = ("nc", zb)
    if key not in _CACHE:
        _CACHE[key] = _build_bass(zero_bias=zb)
    nc = _CACHE[key]
    res = run_bass_kernel_spmd(nc, in_maps, core_ids=list(range(B)))
    outs = np.stack([np.asarray(res.results[b]["out"], np.float32)
                     for b in range(B)], axis=0)
    if not zb:
        outs = outs + np.asarray(inputs["bproj"], np.float32)[None, None, :]
    return outs
